# revision 9
# baseline (speedup 1.0000x reference)
"""Mean-aggregator (GNN message passing) Bass kernel for 8 trn2 NeuronCores.

Algorithm: out[s] = mean over edges e with seg_ids[e]==s of features[neigh_idx[e]].

Sharding: data-parallel over destination segments. Core c owns segments
[c*5120, (c+1)*5120) = 40 aligned blocks of 128 segments. Since seg_ids is
sorted, each core's edges are a contiguous slice. All 8 cores run one
identical SPMD program; all data-dependent structure is padded host-side to
common sizes (maxima over all cores/blocks).

Gather: the f16 feature table is fetched edge-by-edge with the native
dma_gather instruction (256B rows, thousands of rows per instruction, so the
~1us SWDGE fixed cost amortizes away). dma_gather indices are int16, so the
50000-row table is split at row 32768: each block's edges are partitioned
(A: node < 32768, B: node >= 32768), each section padded to a fixed tile
count (KA/KB tiles of 128 edges), and two index streams gather from the two
table halves. Pad slots point at row 0 (always valid) and carry relseg = -1.

Compute per 128-edge tile: DVE builds S[e, s] = (relseg[e] == s) (batched 32
tiles per tensor_tensor via an iota compare); PE accumulates
  sums  += S.T @ X      [128 segs, 128 feats]  (PSUM f32)
  count += S.T @ ones   [128 segs, 1]
across the block's K tiles. Flush: clamp counts to >=1, reciprocal, scale,
DMA the [128, 128] f32 block to DRAM.
"""

import numpy as np

NUM_NODES = 50000
FEAT = 128
NUM_BATCH = 40000
N_CORES = 8
BLOCKS_PER_CORE = 40
SEG_BLOCK = 128
SEGS_PER_CORE = BLOCKS_PER_CORE * SEG_BLOCK  # 5120
SPLIT = 32768  # int16-addressable prefix of the feature table
GROUP = 32  # tiles per S-build tensor_tensor
GB = 8  # blocks covered per gather instruction pair

_program_cache: dict = {}


def _build_program(KA: int, KB: int):
    """Build (and cache) the SPMD Bass program for KA/KB tiles per block."""
    key = (KA, KB)
    if key in _program_cache:
        return _program_cache[key]

    import concourse.bacc as bacc
    import concourse.mybir as mybir
    import concourse.tile as tile

    K = KA + KB
    T = BLOCKS_PER_CORE * K
    TA = BLOCKS_PER_CORE * KA
    GA = -(-TA // GROUP)  # S-build groups, A region
    GBn = -(-(T - TA) // GROUP)  # S-build groups, B region
    CA, CB = GB * KA, GB * KB  # tiles per gather instruction
    NB_GATHER = -(-BLOCKS_PER_CORE // GB)
    f32 = mybir.dt.float32
    f16 = mybir.dt.float16
    i16 = mybir.dt.int16

    nc = bacc.Bacc("TRN2", target_bir_lowering=False, debug=False)
    feat = nc.dram_tensor("features", [NUM_NODES, FEAT], f16, kind="ExternalInput")
    # wrapped int16 gather indices: A-region columns [0, TA*8), B-region after
    idxw = nc.dram_tensor("idxw", [128, T * 8], i16, kind="ExternalInput")
    relseg = nc.dram_tensor("relseg", [128, T], f16, kind="ExternalInput")
    out = nc.dram_tensor("out", [SEGS_PER_CORE, FEAT], f32, kind="ExternalOutput")

    with tile.TileContext(nc) as tc:
        with (
            tc.tile_pool(name="const", bufs=1) as constp,
            tc.tile_pool(name="idx", bufs=1) as idxp,
            tc.tile_pool(name="xa", bufs=2) as xap,
            tc.tile_pool(name="xb", bufs=2) as xbp,
            tc.tile_pool(name="sa", bufs=3) as sap,
            tc.tile_pool(name="sb", bufs=3) as sbp,
            tc.tile_pool(name="fl", bufs=4) as flp,
            tc.tile_pool(name="ps", bufs=3, space="PSUM") as pp,
            tc.tile_pool(name="pc", bufs=3, space="PSUM") as pcp,
        ):
            idxw_sb = idxp.tile([128, T * 8], i16)
            relseg_sb = idxp.tile([128, T], f16)
            nc.sync.dma_start(idxw_sb[:], idxw[:])
            nc.sync.dma_start(relseg_sb[:], relseg[:])

            # iota_rep[p, j*128 + s] = s for j in [0, GROUP)
            iota_i = constp.tile([128, GROUP * 128], i16)
            iota_f = constp.tile([128, GROUP * 128], f16)
            nc.gpsimd.iota(
                iota_i[:], pattern=[[0, GROUP], [1, 128]], base=0, channel_multiplier=0
            )
            nc.vector.tensor_copy(iota_f[:], iota_i[:])
            ones_col = constp.tile([128, 1], f16)
            nc.vector.memset(ones_col[:], 1.0)

            # S-build groups, separate per region so each pool's allocation
            # order matches its consumption order (blocks ascending)
            def build_s(pool, tag, base, ntiles, g):
                t0 = base + g * GROUP
                w = min(GROUP, base + ntiles - t0)
                st = pool.tile([128, GROUP * 128], f16, tag=tag)
                nc.vector.tensor_tensor(
                    out=st[:, : w * 128].rearrange("p (j s) -> p j s", s=128),
                    in0=relseg_sb[:, t0 : t0 + w].to_broadcast([128, w, 128]),
                    in1=iota_f[:, : w * 128].rearrange("p (j s) -> p j s", s=128),
                    op=mybir.AluOpType.is_equal,
                )
                return st

            sts_a = [build_s(sap, "sta", 0, TA, g) for g in range(GA)]
            sts_b = [build_s(sbp, "stb", TA, T - TA, g) for g in range(GBn)]

            # gathers: pair (A, B) per GB blocks, interleaved with consumption
            xa_tiles: list = []
            xb_tiles: list = []
            for i in range(NB_GATHER):
                wa = min(CA, TA - i * CA)
                xa = xap.tile([128, CA * 128], f16, tag="xa")
                nc.gpsimd.dma_gather(
                    out_ap=xa[:, : wa * 128].rearrange("p (c e) -> p c e", e=128),
                    in_ap=feat[:SPLIT, :],
                    idxs_ap=idxw_sb[:, i * CA * 8 : (i * CA + wa) * 8],
                    num_idxs=wa * 128,
                    num_idxs_reg=wa * 128,
                    elem_size=FEAT,
                    single_packet=False,
                )
                xa_tiles.append(xa)
                if KB > 0:
                    wb = min(CB, (T - TA) - i * CB)
                    xb = xbp.tile([128, CB * 128], f16, tag="xb")
                    nc.gpsimd.dma_gather(
                        out_ap=xb[:, : wb * 128].rearrange("p (c e) -> p c e", e=128),
                        in_ap=feat[SPLIT:, :],
                        idxs_ap=idxw_sb[
                            :, (TA + i * CB) * 8 : (TA + i * CB + wb) * 8
                        ],
                        num_idxs=wb * 128,
                        num_idxs_reg=wb * 128,
                        elem_size=FEAT,
                        single_packet=False,
                    )
                    xb_tiles.append(xb)

            def slot_rhs(st_idx):
                """xt AP [128, 128] for slot-tile st_idx."""
                if st_idx < TA:
                    i, pos = divmod(st_idx, CA)
                    return xa_tiles[i][:, pos * 128 : (pos + 1) * 128]
                i, pos = divmod(st_idx - TA, CB)
                return xb_tiles[i][:, pos * 128 : (pos + 1) * 128]

            for b in range(BLOCKS_PER_CORE):
                ps = pp.tile([128, FEAT], f32, space="PSUM")
                pcnt = pcp.tile([128, 1], f32, space="PSUM")
                for j in range(K):
                    if j < KA:
                        st_idx = b * KA + j
                        g, jj = divmod(st_idx, GROUP)
                        lhsT = sts_a[g][:, jj * 128 : (jj + 1) * 128]
                    else:
                        st_idx = TA + b * KB + (j - KA)
                        g, jj = divmod(st_idx - TA, GROUP)
                        lhsT = sts_b[g][:, jj * 128 : (jj + 1) * 128]
                    rhs = slot_rhs(st_idx)
                    nc.tensor.matmul(
                        ps[:], lhsT=lhsT, rhs=rhs,
                        start=(j == 0), stop=(j == K - 1),
                    )
                    nc.tensor.matmul(
                        pcnt[:], lhsT=lhsT, rhs=ones_col[:],
                        start=(j == 0), stop=(j == K - 1),
                    )
                cnt = flp.tile([128, 1], f32, tag="cnt")
                rcnt = flp.tile([128, 1], f32, tag="rcnt")
                ob = flp.tile([128, FEAT], f32, tag="ob")
                nc.vector.tensor_scalar_max(cnt[:], pcnt[:], 1.0)
                nc.vector.reciprocal(rcnt[:], cnt[:])
                nc.vector.tensor_scalar_mul(ob[:], ps[:], rcnt[:])
                nc.sync.dma_start(out[b * 128 : (b + 1) * 128, :], ob[:])

    nc.compile()
    _program_cache[key] = nc
    return nc


def _prepare_inputs(features, neigh_idx, seg_ids):
    """Shard edges by segment block; within each block partition edges into
    A (node < SPLIT) then B, pad sections to KA/KB tiles. Returns
    (features f16, per-core idxw [128, T*8] i16, per-core relseg [128, T] f16,
    KA, KB)."""
    n_blocks = N_CORES * BLOCKS_PER_CORE
    bases = np.arange(n_blocks + 1, dtype=np.int64) * SEG_BLOCK
    bnd = np.searchsorted(seg_ids, bases)

    nidx64 = np.asarray(neigh_idx)
    seg64 = np.asarray(seg_ids)

    # per-block A/B edge lists
    blocks = []
    maxa = maxb = 0
    for i in range(n_blocks):
        lo, hi = bnd[i], bnd[i + 1]
        nodes = nidx64[lo:hi]
        rs = (seg64[lo:hi] - bases[i]).astype(np.float16)
        a_mask = nodes < SPLIT
        na = int(a_mask.sum())
        nb = len(nodes) - na
        blocks.append((nodes[a_mask], rs[a_mask], nodes[~a_mask], rs[~a_mask]))
        maxa = max(maxa, na)
        maxb = max(maxb, nb)
    KA = max(1, -(-maxa // 128))
    KB = -(-maxb // 128)
    K = KA + KB
    T = BLOCKS_PER_CORE * K
    TA = BLOCKS_PER_CORE * KA

    idxA = np.zeros((N_CORES, TA * 128), dtype=np.int16)
    idxB = np.zeros((N_CORES, (T - TA) * 128), dtype=np.int16)
    relseg_slots = np.full((N_CORES, T * 128), -1.0, dtype=np.float16)
    for i in range(n_blocks):
        c, b = divmod(i, BLOCKS_PER_CORE)
        an, ar, bn, br = blocks[i]
        oa = b * KA * 128
        idxA[c, oa : oa + len(an)] = an.astype(np.int16)
        relseg_slots[c, oa : oa + len(ar)] = ar
        if KB:
            ob = b * KB * 128
            idxB[c, ob : ob + len(bn)] = (bn - SPLIT).astype(np.int16)
            relseg_slots[c, TA * 128 + ob : TA * 128 + ob + len(br)] = br

    # wrap indices: flat i -> [i % 16, i // 16], replicated to 128 partitions
    def wrap(a):
        w = a.reshape(-1, 16).T  # [16, n/16]
        return np.tile(w, (8, 1))  # [128, n/16]

    idxw = [
        np.ascontiguousarray(
            np.concatenate([wrap(idxA[c]), wrap(idxB[c])], axis=1)
            if KB
            else wrap(idxA[c])
        )
        for c in range(N_CORES)
    ]
    relseg_t = [
        np.ascontiguousarray(a.reshape(T, 128).T) for a in relseg_slots
    ]
    feat16 = np.ascontiguousarray(features.astype(np.float16))
    return feat16, idxw, relseg_t, KA, KB


LAST_RESULT = None


def kernel(features, neigh_idx, seg_ids, num_batch, _trace=False):
    global LAST_RESULT
    from concourse.bass_utils import run_bass_kernel_spmd

    features = np.asarray(features, dtype=np.float32)
    neigh_idx = np.asarray(neigh_idx)
    seg_ids = np.asarray(seg_ids)
    nb = int(num_batch)
    assert nb == NUM_BATCH, nb
    assert features.shape == (NUM_NODES, FEAT), features.shape

    feat16, idxw, relseg_t, KA, KB = _prepare_inputs(features, neigh_idx, seg_ids)
    nc = _build_program(KA, KB)

    in_maps = [
        {"features": feat16, "idxw": idxw[c], "relseg": relseg_t[c]}
        for c in range(N_CORES)
    ]
    res = run_bass_kernel_spmd(
        nc, in_maps, core_ids=list(range(N_CORES)), trace=_trace
    )
    LAST_RESULT = res

    out = np.empty((NUM_BATCH, FEAT), dtype=np.float32)
    for c in range(N_CORES):
        lo = c * SEGS_PER_CORE
        hi = min(lo + SEGS_PER_CORE, NUM_BATCH)
        if hi > lo:
            out[lo:hi] = res.results[c]["out"][: hi - lo]
    return out


# revision 12
# speedup vs baseline: 4.1785x; 4.1785x over previous
"""Mean-aggregator (GNN message passing) Bass kernel for 8 trn2 NeuronCores.

Algorithm: out[s] = mean over edges e with seg_ids[e]==s of features[neigh_idx[e]].

Sharding: data-parallel over destination segments. Core c owns segments
[c*5120, (c+1)*5120) = 40 aligned blocks of 128 segments. Since seg_ids is
sorted, each core's edges are a contiguous slice. All 8 cores run one
identical SPMD program; all data-dependent structure is padded host-side to
common sizes (maxima over all cores/blocks).

Gather: the f16 feature table is fetched edge-by-edge with the native
dma_gather instruction (256B rows, thousands of rows per instruction,
spread round-robin over the 4 SWDGE queues so descriptor generation runs on
all four Q7 core pairs in parallel). dma_gather indices are int16, so the
50000-row table is split at a host-tuned row SPLIT < 32768: each block's
edges are partitioned (A: node < SPLIT, B: node >= SPLIT), each section
padded to a fixed tile count (KA/KB tiles of 128 edges), and two index
streams gather from the two table halves. Pad slots point at row 0 (always
valid) and carry relseg = -1.

Compute per 128-edge tile: DVE builds S[e, s] = (relseg[e] == s) (batched 32
tiles per tensor_tensor via an iota compare); PE accumulates
  sums += S.T @ X      [128 segs, 128 feats]  (PSUM f32)
across the block's K tiles. Segment counts are host-side index preprocessing
(bincount of seg_ids); the flush scales the PSUM block by the preloaded
reciprocal counts and DMAs the [128, 128] f32 block out.
"""

import numpy as np

NUM_NODES = 50000
FEAT = 128
NUM_BATCH = 40000
N_CORES = 8
BLOCKS_PER_CORE = 40
SEG_BLOCK = 128
SEGS_PER_CORE = BLOCKS_PER_CORE * SEG_BLOCK  # 5120
GROUP = 32  # tiles per S-build tensor_tensor
GB = 4  # blocks covered per gather instruction pair

_program_cache: dict = {}


def _build_program(KA: int, KB: int, split: int):
    """Build (and cache) the SPMD Bass program for KA/KB tiles per block."""
    key = (KA, KB, split)
    if key in _program_cache:
        return _program_cache[key]

    import concourse.bacc as bacc
    import concourse.mybir as mybir
    import concourse.tile as tile

    K = KA + KB
    T = BLOCKS_PER_CORE * K
    TA = BLOCKS_PER_CORE * KA
    GA = -(-TA // GROUP)  # S-build groups, A region
    GBn = -(-(T - TA) // GROUP)  # S-build groups, B region
    CA, CB = GB * KA, GB * KB  # tiles per gather instruction pair
    NB_GATHER = -(-BLOCKS_PER_CORE // GB)
    f32 = mybir.dt.float32
    f16 = mybir.dt.float16
    i16 = mybir.dt.int16

    nc = bacc.Bacc(
        "TRN2", target_bir_lowering=False, debug=False, num_swdge_queues=4
    )
    feat = nc.dram_tensor("features", [NUM_NODES, FEAT], f16, kind="ExternalInput")
    # wrapped int16 gather indices: A-region columns [0, TA*8), B-region after
    idxw = nc.dram_tensor("idxw", [128, T * 8], i16, kind="ExternalInput")
    relseg = nc.dram_tensor("relseg", [128, T], f16, kind="ExternalInput")
    # rc[p, b] = 1/max(count, 1) for segment b*128+p of this core
    rc = nc.dram_tensor("rc", [128, BLOCKS_PER_CORE], f32, kind="ExternalInput")
    out = nc.dram_tensor("out", [SEGS_PER_CORE, FEAT], f32, kind="ExternalOutput")

    with tile.TileContext(nc) as tc:
        with (
            tc.tile_pool(name="const", bufs=1) as constp,
            tc.tile_pool(name="idx", bufs=1) as idxp,
            tc.tile_pool(name="xa", bufs=3) as xap,
            tc.tile_pool(name="xb", bufs=3) as xbp,
            tc.tile_pool(name="sa", bufs=3) as sap,
            tc.tile_pool(name="sb", bufs=3) as sbp,
            tc.tile_pool(name="fl", bufs=4) as flp,
            tc.tile_pool(name="ps", bufs=6, space="PSUM") as pp,
        ):
            idxw_sb = idxp.tile([128, T * 8], i16)
            relseg_sb = idxp.tile([128, T], f16)
            rc_sb = idxp.tile([128, BLOCKS_PER_CORE], f32)
            nc.sync.dma_start(idxw_sb[:], idxw[:])
            nc.sync.dma_start(relseg_sb[:], relseg[:])
            nc.sync.dma_start(rc_sb[:], rc[:])

            # iota_rep[p, j*128 + s] = s for j in [0, GROUP)
            iota_i = constp.tile([128, GROUP * 128], i16)
            iota_f = constp.tile([128, GROUP * 128], f16)
            nc.gpsimd.iota(
                iota_i[:], pattern=[[0, GROUP], [1, 128]], base=0, channel_multiplier=0
            )
            nc.vector.tensor_copy(iota_f[:], iota_i[:])

            # S-build groups, separate per region so each pool's allocation
            # order matches its consumption order (blocks ascending)
            def build_s(pool, tag, base, ntiles, g):
                t0 = base + g * GROUP
                w = min(GROUP, base + ntiles - t0)
                st = pool.tile([128, GROUP * 128], f16, tag=tag)
                nc.vector.tensor_tensor(
                    out=st[:, : w * 128].rearrange("p (j s) -> p j s", s=128),
                    in0=relseg_sb[:, t0 : t0 + w].to_broadcast([128, w, 128]),
                    in1=iota_f[:, : w * 128].rearrange("p (j s) -> p j s", s=128),
                    op=mybir.AluOpType.is_equal,
                )
                return st

            sts_a = [build_s(sap, "sta", 0, TA, g) for g in range(GA)]
            sts_b = [build_s(sbp, "stb", TA, T - TA, g) for g in range(GBn)]

            # gathers: pair (A, B) per GB blocks, each split across the 4
            # SWDGE queues (descriptor generation runs on cpu pair ==
            # queue_num, so 4 queues generate in parallel)
            qrr = [0]

            def gather_tiles(xtile, w, col0, table_ap):
                nq = 4
                step = -(-w // nq)
                for q0 in range(0, w, step):
                    ww = min(step, w - q0)
                    nc.gpsimd.dma_gather(
                        out_ap=xtile[:, q0 * 128 : (q0 + ww) * 128].rearrange(
                            "p (c e) -> p c e", e=128
                        ),
                        in_ap=table_ap,
                        idxs_ap=idxw_sb[:, (col0 + q0) * 8 : (col0 + q0 + ww) * 8],
                        num_idxs=ww * 128,
                        num_idxs_reg=ww * 128,
                        elem_size=FEAT,
                        single_packet=False,
                        queue_num=qrr[0] % 4,
                    )
                    qrr[0] += 1

            xa_tiles: list = []
            xb_tiles: list = []
            for i in range(NB_GATHER):
                wa = min(CA, TA - i * CA)
                xa = xap.tile([128, CA * 128], f16, tag="xa")
                gather_tiles(xa, wa, i * CA, feat[:split, :])
                xa_tiles.append(xa)
                if KB > 0:
                    wb = min(CB, (T - TA) - i * CB)
                    xb = xbp.tile([128, CB * 128], f16, tag="xb")
                    gather_tiles(xb, wb, TA + i * CB, feat[split:, :])
                    xb_tiles.append(xb)

            def slot_rhs(st_idx):
                if st_idx < TA:
                    i, pos = divmod(st_idx, CA)
                    return xa_tiles[i][:, pos * 128 : (pos + 1) * 128]
                i, pos = divmod(st_idx - TA, CB)
                return xb_tiles[i][:, pos * 128 : (pos + 1) * 128]

            for b in range(BLOCKS_PER_CORE):
                ps = pp.tile([128, FEAT], f32, space="PSUM")
                for j in range(K):
                    if j < KA:
                        st_idx = b * KA + j
                        g, jj = divmod(st_idx, GROUP)
                        lhsT = sts_a[g][:, jj * 128 : (jj + 1) * 128]
                    else:
                        st_idx = TA + b * KB + (j - KA)
                        g, jj = divmod(st_idx - TA, GROUP)
                        lhsT = sts_b[g][:, jj * 128 : (jj + 1) * 128]
                    nc.tensor.matmul(
                        ps[:], lhsT=lhsT, rhs=slot_rhs(st_idx),
                        start=(j == 0), stop=(j == K - 1),
                    )
                ob = flp.tile([128, FEAT], f32, tag="ob")
                nc.vector.tensor_scalar_mul(ob[:], ps[:], rc_sb[:, b : b + 1])
                nc.sync.dma_start(out[b * 128 : (b + 1) * 128, :], ob[:])

    nc.compile()
    _program_cache[key] = nc
    return nc


def _prepare_inputs(features, neigh_idx, seg_ids):
    """Shard edges by segment block; within each block partition edges into
    A (node < split) then B, pad sections to KA/KB tiles. The split point is
    tuned to minimize total padded tiles. Returns (features f16, per-core
    idxw [128, T*8] i16, per-core relseg [128, T] f16, per-core rc [128, 40]
    f32, KA, KB, split)."""
    n_blocks = N_CORES * BLOCKS_PER_CORE
    bases = np.arange(n_blocks + 1, dtype=np.int64) * SEG_BLOCK
    bnd = np.searchsorted(seg_ids, bases)

    nidx64 = np.asarray(neigh_idx)
    seg64 = np.asarray(seg_ids)

    # tune the table split point: minimize KA+KB over candidates
    lo = max(0, NUM_NODES - 32768)
    candidates = np.linspace(lo + 256, 32768, 12).astype(np.int64)
    block_nodes = [np.sort(nidx64[bnd[i] : bnd[i + 1]]) for i in range(n_blocks)]
    sizes = np.array([len(x) for x in block_nodes])
    best = None
    for s in candidates:
        na = np.array([np.searchsorted(x, s) for x in block_nodes])
        nb = sizes - na
        ka = max(1, -(-int(na.max()) // 128))
        kb = -(-int(nb.max()) // 128)
        if best is None or ka + kb < best[0] + best[1]:
            best = (ka, kb, int(s))
    KA, KB, split = best
    K = KA + KB
    T = BLOCKS_PER_CORE * K
    TA = BLOCKS_PER_CORE * KA

    idxA = np.zeros((N_CORES, TA * 128), dtype=np.int16)
    idxB = np.zeros((N_CORES, (T - TA) * 128), dtype=np.int16)
    relseg_slots = np.full((N_CORES, T * 128), -1.0, dtype=np.float16)
    for i in range(n_blocks):
        c, b = divmod(i, BLOCKS_PER_CORE)
        lo_, hi_ = bnd[i], bnd[i + 1]
        nodes = nidx64[lo_:hi_]
        rs = (seg64[lo_:hi_] - bases[i]).astype(np.float16)
        a_mask = nodes < split
        an, ar = nodes[a_mask], rs[a_mask]
        bn, br = nodes[~a_mask], rs[~a_mask]
        oa = b * KA * 128
        idxA[c, oa : oa + len(an)] = an.astype(np.int16)
        relseg_slots[c, oa : oa + len(ar)] = ar
        if KB:
            ob = b * KB * 128
            idxB[c, ob : ob + len(bn)] = (bn - split).astype(np.int16)
            relseg_slots[c, TA * 128 + ob : TA * 128 + ob + len(br)] = br

    # wrap indices: flat i -> [i % 16, i // 16], replicated to 128 partitions
    def wrap(a):
        w = a.reshape(-1, 16).T
        return np.tile(w, (8, 1))

    idxw = [
        np.ascontiguousarray(
            np.concatenate([wrap(idxA[c]), wrap(idxB[c])], axis=1)
            if KB
            else wrap(idxA[c])
        )
        for c in range(N_CORES)
    ]
    relseg_t = [np.ascontiguousarray(a.reshape(T, 128).T) for a in relseg_slots]

    counts = np.bincount(seg64, minlength=N_CORES * SEGS_PER_CORE).astype(np.float64)
    rcg = (1.0 / np.maximum(counts, 1.0)).astype(np.float32)
    rc = [
        np.ascontiguousarray(
            rcg[c * SEGS_PER_CORE : (c + 1) * SEGS_PER_CORE]
            .reshape(BLOCKS_PER_CORE, 128)
            .T
        )
        for c in range(N_CORES)
    ]
    feat16 = np.ascontiguousarray(features.astype(np.float16))
    return feat16, idxw, relseg_t, rc, KA, KB, split


LAST_RESULT = None


def kernel(features, neigh_idx, seg_ids, num_batch, _trace=False):
    global LAST_RESULT
    from concourse.bass_utils import run_bass_kernel_spmd

    features = np.asarray(features, dtype=np.float32)
    neigh_idx = np.asarray(neigh_idx)
    seg_ids = np.asarray(seg_ids)
    nb = int(num_batch)
    assert nb == NUM_BATCH, nb
    assert features.shape == (NUM_NODES, FEAT), features.shape

    feat16, idxw, relseg_t, rc, KA, KB, split = _prepare_inputs(
        features, neigh_idx, seg_ids
    )
    nc = _build_program(KA, KB, split)

    in_maps = [
        {"features": feat16, "idxw": idxw[c], "relseg": relseg_t[c], "rc": rc[c]}
        for c in range(N_CORES)
    ]
    res = run_bass_kernel_spmd(
        nc, in_maps, core_ids=list(range(N_CORES)), trace=_trace
    )
    LAST_RESULT = res

    out = np.empty((NUM_BATCH, FEAT), dtype=np.float32)
    for c in range(N_CORES):
        lo = c * SEGS_PER_CORE
        hi = min(lo + SEGS_PER_CORE, NUM_BATCH)
        if hi > lo:
            out[lo:hi] = res.results[c]["out"][: hi - lo]
    return out


# revision 14
# speedup vs baseline: 4.3549x; 1.0422x over previous
"""Mean-aggregator (GNN message passing) Bass kernel for 8 trn2 NeuronCores.

Algorithm: out[s] = mean over edges e with seg_ids[e]==s of features[neigh_idx[e]].

Sharding: data-parallel over destination segments. Core c owns segments
[c*5120, (c+1)*5120) = 40 aligned blocks of 128 segments. Since seg_ids is
sorted, each core's edges are a contiguous slice. All 8 cores run one
identical SPMD program; all data-dependent structure is padded host-side to
common sizes (maxima over all cores/blocks).

Gather: the f16 feature table is fetched edge-by-edge with the native
dma_gather instruction (256B rows, thousands of rows per instruction,
spread round-robin over the 4 SWDGE queues so descriptor generation runs on
all four Q7 core pairs in parallel). dma_gather indices are int16, so the
50000-row table is split at a host-tuned row SPLIT < 32768: each block's
edges are partitioned (A: node < SPLIT, B: node >= SPLIT), each section
padded to a fixed tile count (KA/KB tiles of 128 edges), and two index
streams gather from the two table halves. Pad slots point at row 0 (always
valid) and carry relseg = -1.

Compute per 128-edge tile: DVE builds S[e, s] = (relseg[e] == s) (batched 32
tiles per tensor_tensor via an iota compare); PE accumulates
  sums += S.T @ X      [128 segs, 128 feats]  (PSUM f32)
across the block's K tiles. Segment counts are host-side index preprocessing
(bincount of seg_ids); the flush scales the PSUM block by the preloaded
reciprocal counts and DMAs the [128, 128] f32 block out.
"""

import numpy as np

NUM_NODES = 50000
FEAT = 128
NUM_BATCH = 40000
N_CORES = 8
BLOCKS_PER_CORE = 40
SEG_BLOCK = 128
SEGS_PER_CORE = BLOCKS_PER_CORE * SEG_BLOCK  # 5120
GROUP = 32  # tiles per S-build tensor_tensor
GB = 4  # blocks covered per gather instruction pair

_program_cache: dict = {}


def _build_program(KA: int, KB: int, split: int):
    """Build (and cache) the SPMD Bass program for KA/KB tiles per block."""
    key = (KA, KB, split)
    if key in _program_cache:
        return _program_cache[key]

    import concourse.bacc as bacc
    import concourse.mybir as mybir
    import concourse.tile as tile

    K = KA + KB
    T = BLOCKS_PER_CORE * K
    TA = BLOCKS_PER_CORE * KA
    GA = -(-TA // GROUP)  # S-build groups, A region
    GBn = -(-(T - TA) // GROUP)  # S-build groups, B region
    CA, CB = GB * KA, GB * KB  # tiles per gather instruction pair
    NB_GATHER = -(-BLOCKS_PER_CORE // GB)
    f32 = mybir.dt.float32
    f16 = mybir.dt.float16
    i16 = mybir.dt.int16

    nc = bacc.Bacc(
        "TRN2", target_bir_lowering=False, debug=False, num_swdge_queues=4
    )
    feat = nc.dram_tensor("features", [NUM_NODES, FEAT], f16, kind="ExternalInput")
    # wrapped int16 gather indices: A-region columns [0, TA*8), B-region after
    idxw = nc.dram_tensor("idxw", [128, T * 8], i16, kind="ExternalInput")
    relseg = nc.dram_tensor("relseg", [128, T], f16, kind="ExternalInput")
    # rc[p, b] = 1/max(count, 1) for segment b*128+p of this core
    rc = nc.dram_tensor("rc", [128, BLOCKS_PER_CORE], f32, kind="ExternalInput")
    out = nc.dram_tensor("out", [SEGS_PER_CORE, FEAT], f32, kind="ExternalOutput")

    with tile.TileContext(nc) as tc:
        with (
            tc.tile_pool(name="const", bufs=1) as constp,
            tc.tile_pool(name="idx", bufs=1) as idxp,
            tc.tile_pool(name="xa", bufs=3) as xap,
            tc.tile_pool(name="xb", bufs=3) as xbp,
            tc.tile_pool(name="sa", bufs=3) as sap,
            tc.tile_pool(name="sb", bufs=3) as sbp,
            tc.tile_pool(name="fl", bufs=4) as flp,
            tc.tile_pool(name="ps", bufs=3, space="PSUM") as pp,
        ):
            idxw_sb = idxp.tile([128, T * 8], i16)
            relseg_sb = idxp.tile([128, T], f16)
            rc_sb = idxp.tile([128, BLOCKS_PER_CORE], f32)
            nc.sync.dma_start(idxw_sb[:], idxw[:])
            nc.sync.dma_start(relseg_sb[:], relseg[:])
            nc.sync.dma_start(rc_sb[:], rc[:])

            # iota_rep[p, j*128 + s] = s for j in [0, GROUP)
            iota_i = constp.tile([128, GROUP * 128], i16)
            iota_f = constp.tile([128, GROUP * 128], f16)
            nc.gpsimd.iota(
                iota_i[:], pattern=[[0, GROUP], [1, 128]], base=0, channel_multiplier=0
            )
            nc.vector.tensor_copy(iota_f[:], iota_i[:])

            # S-build groups, separate per region so each pool's allocation
            # order matches its consumption order (blocks ascending)
            def build_s(pool, tag, base, ntiles, g):
                t0 = base + g * GROUP
                w = min(GROUP, base + ntiles - t0)
                st = pool.tile([128, GROUP * 128], f16, tag=tag)
                nc.vector.tensor_tensor(
                    out=st[:, : w * 128].rearrange("p (j s) -> p j s", s=128),
                    in0=relseg_sb[:, t0 : t0 + w].to_broadcast([128, w, 128]),
                    in1=iota_f[:, : w * 128].rearrange("p (j s) -> p j s", s=128),
                    op=mybir.AluOpType.is_equal,
                )
                return st

            sts_a = [build_s(sap, "sta", 0, TA, g) for g in range(GA)]
            sts_b = [build_s(sbp, "stb", TA, T - TA, g) for g in range(GBn)]

            # gathers: pair (A, B) per GB blocks, each split across the 4
            # SWDGE queues (descriptor generation runs on cpu pair ==
            # queue_num, so 4 queues generate in parallel)
            qrr = [0]

            def gather_tiles(xtile, w, col0, table_ap):
                nq = 4
                step = -(-w // nq)
                for q0 in range(0, w, step):
                    ww = min(step, w - q0)
                    nc.gpsimd.dma_gather(
                        out_ap=xtile[:, q0 * 128 : (q0 + ww) * 128].rearrange(
                            "p (c e) -> p c e", e=128
                        ),
                        in_ap=table_ap,
                        idxs_ap=idxw_sb[:, (col0 + q0) * 8 : (col0 + q0 + ww) * 8],
                        num_idxs=ww * 128,
                        num_idxs_reg=ww * 128,
                        elem_size=FEAT,
                        single_packet=False,
                        queue_num=qrr[0] % 4,
                    )
                    qrr[0] += 1

            xa_tiles: list = []
            xb_tiles: list = []
            for i in range(NB_GATHER):
                wa = min(CA, TA - i * CA)
                xa = xap.tile([128, CA * 128], f16, tag="xa")
                gather_tiles(xa, wa, i * CA, feat[:split, :])
                xa_tiles.append(xa)
                if KB > 0:
                    wb = min(CB, (T - TA) - i * CB)
                    xb = xbp.tile([128, CB * 128], f16, tag="xb")
                    gather_tiles(xb, wb, TA + i * CB, feat[split:, :])
                    xb_tiles.append(xb)

            def slot_rhs(st_idx):
                if st_idx < TA:
                    i, pos = divmod(st_idx, CA)
                    return xa_tiles[i][:, pos * 128 : (pos + 1) * 128]
                i, pos = divmod(st_idx - TA, CB)
                return xb_tiles[i][:, pos * 128 : (pos + 1) * 128]

            def block_lhsT(b, j):
                if j < KA:
                    st_idx = b * KA + j
                    g, jj = divmod(st_idx, GROUP)
                    return sts_a[g][:, jj * 128 : (jj + 1) * 128], st_idx
                st_idx = TA + b * KB + (j - KA)
                g, jj = divmod(st_idx - TA, GROUP)
                return sts_b[g][:, jj * 128 : (jj + 1) * 128], st_idx

            # blocks processed in pairs with their matmul chains interleaved:
            # consecutive PE matmuls hit different PSUM banks, letting the
            # array overlap one matmul's drain with the next one's fill
            for b0 in range(0, BLOCKS_PER_CORE, 2):
                pse = pp.tile([128, FEAT], f32, space="PSUM", tag="pse")
                pso = pp.tile([128, FEAT], f32, space="PSUM", tag="pso")
                for j in range(K):
                    for b, ps in ((b0, pse), (b0 + 1, pso)):
                        lhsT, st_idx = block_lhsT(b, j)
                        nc.tensor.matmul(
                            ps[:], lhsT=lhsT, rhs=slot_rhs(st_idx),
                            start=(j == 0), stop=(j == K - 1),
                        )
                for b, ps in ((b0, pse), (b0 + 1, pso)):
                    ob = flp.tile([128, FEAT], f32, tag="ob")
                    nc.vector.tensor_scalar_mul(ob[:], ps[:], rc_sb[:, b : b + 1])
                    nc.sync.dma_start(out[b * 128 : (b + 1) * 128, :], ob[:])

    nc.compile()
    _program_cache[key] = nc
    return nc


def _prepare_inputs(features, neigh_idx, seg_ids):
    """Shard edges by segment block; within each block partition edges into
    A (node < split) then B, pad sections to KA/KB tiles. The split point is
    tuned to minimize total padded tiles. Returns (features f16, per-core
    idxw [128, T*8] i16, per-core relseg [128, T] f16, per-core rc [128, 40]
    f32, KA, KB, split)."""
    n_blocks = N_CORES * BLOCKS_PER_CORE
    bases = np.arange(n_blocks + 1, dtype=np.int64) * SEG_BLOCK
    bnd = np.searchsorted(seg_ids, bases)

    nidx64 = np.asarray(neigh_idx)
    seg64 = np.asarray(seg_ids)

    # tune the table split point: minimize KA+KB over candidates
    lo = max(0, NUM_NODES - 32768)
    candidates = np.linspace(lo + 256, 32768, 12).astype(np.int64)
    block_nodes = [np.sort(nidx64[bnd[i] : bnd[i + 1]]) for i in range(n_blocks)]
    sizes = np.array([len(x) for x in block_nodes])
    best = None
    for s in candidates:
        na = np.array([np.searchsorted(x, s) for x in block_nodes])
        nb = sizes - na
        ka = max(1, -(-int(na.max()) // 128))
        kb = -(-int(nb.max()) // 128)
        if best is None or ka + kb < best[0] + best[1]:
            best = (ka, kb, int(s))
    KA, KB, split = best
    K = KA + KB
    T = BLOCKS_PER_CORE * K
    TA = BLOCKS_PER_CORE * KA

    idxA = np.zeros((N_CORES, TA * 128), dtype=np.int16)
    idxB = np.zeros((N_CORES, (T - TA) * 128), dtype=np.int16)
    relseg_slots = np.full((N_CORES, T * 128), -1.0, dtype=np.float16)
    for i in range(n_blocks):
        c, b = divmod(i, BLOCKS_PER_CORE)
        lo_, hi_ = bnd[i], bnd[i + 1]
        nodes = nidx64[lo_:hi_]
        rs = (seg64[lo_:hi_] - bases[i]).astype(np.float16)
        a_mask = nodes < split
        an, ar = nodes[a_mask], rs[a_mask]
        bn, br = nodes[~a_mask], rs[~a_mask]
        oa = b * KA * 128
        idxA[c, oa : oa + len(an)] = an.astype(np.int16)
        relseg_slots[c, oa : oa + len(ar)] = ar
        if KB:
            ob = b * KB * 128
            idxB[c, ob : ob + len(bn)] = (bn - split).astype(np.int16)
            relseg_slots[c, TA * 128 + ob : TA * 128 + ob + len(br)] = br

    # wrap indices: flat i -> [i % 16, i // 16], replicated to 128 partitions
    def wrap(a):
        w = a.reshape(-1, 16).T
        return np.tile(w, (8, 1))

    idxw = [
        np.ascontiguousarray(
            np.concatenate([wrap(idxA[c]), wrap(idxB[c])], axis=1)
            if KB
            else wrap(idxA[c])
        )
        for c in range(N_CORES)
    ]
    relseg_t = [np.ascontiguousarray(a.reshape(T, 128).T) for a in relseg_slots]

    counts = np.bincount(seg64, minlength=N_CORES * SEGS_PER_CORE).astype(np.float64)
    rcg = (1.0 / np.maximum(counts, 1.0)).astype(np.float32)
    rc = [
        np.ascontiguousarray(
            rcg[c * SEGS_PER_CORE : (c + 1) * SEGS_PER_CORE]
            .reshape(BLOCKS_PER_CORE, 128)
            .T
        )
        for c in range(N_CORES)
    ]
    feat16 = np.ascontiguousarray(features.astype(np.float16))
    return feat16, idxw, relseg_t, rc, KA, KB, split


LAST_RESULT = None


def kernel(features, neigh_idx, seg_ids, num_batch, _trace=False):
    global LAST_RESULT
    from concourse.bass_utils import run_bass_kernel_spmd

    features = np.asarray(features, dtype=np.float32)
    neigh_idx = np.asarray(neigh_idx)
    seg_ids = np.asarray(seg_ids)
    nb = int(num_batch)
    assert nb == NUM_BATCH, nb
    assert features.shape == (NUM_NODES, FEAT), features.shape

    feat16, idxw, relseg_t, rc, KA, KB, split = _prepare_inputs(
        features, neigh_idx, seg_ids
    )
    nc = _build_program(KA, KB, split)

    in_maps = [
        {"features": feat16, "idxw": idxw[c], "relseg": relseg_t[c], "rc": rc[c]}
        for c in range(N_CORES)
    ]
    res = run_bass_kernel_spmd(
        nc, in_maps, core_ids=list(range(N_CORES)), trace=_trace
    )
    LAST_RESULT = res

    out = np.empty((NUM_BATCH, FEAT), dtype=np.float32)
    for c in range(N_CORES):
        lo = c * SEGS_PER_CORE
        hi = min(lo + SEGS_PER_CORE, NUM_BATCH)
        if hi > lo:
            out[lo:hi] = res.results[c]["out"][: hi - lo]
    return out
